# revision 1
# baseline (speedup 1.0000x reference)
"""V2: engine-rebalanced AttentionBlock kernel (see kernel.py baseline).

Changes vs baseline:
- softmax exp split: ~2/3 of tiles exact exp on ACT, ~1/3 Schraudolph
  bit-trick (f32->int16 affine, bitcast bf16) on DVE
- qk bias-add + vt psum->sbuf copies moved to ACT (activation Copy)
- qkv weight scaling + attn-normalize mult moved to Pool (gpsimd)
- vt ones-column init via Pool memset instead of DMA
"""
import contextlib
import numpy as np
import concourse.bacc as bacc
import concourse.tile as tile
from concourse import mybir

F32 = mybir.dt.float32
F32R = mybir.dt.float32r
BF16 = mybir.dt.bfloat16
I16 = mybir.dt.int16
AF = mybir.ActivationFunctionType
ALU = mybir.AluOpType

B_PER_CORE = 2
EPS = 1e-5

# Schraudolph exp: exp(0.125*s) ~= bf16_bits(round(A*s + B))
LOG2E = 1.4426950408889634
SCH_A = 0.125 * 128.0 * LOG2E
SCH_B = 127.0 * 128.0 - 11.0


def build(nbatch=B_PER_CORE, loop_reps=1):
    nc = bacc.Bacc("TRN2", target_bir_lowering=False, debug=False)

    x_d = nc.dram_tensor("x", [nbatch, 256, 32, 32], F32R, kind="ExternalInput")
    wqkvT_d = nc.dram_tensor("wqkvT", [256, 768], F32R, kind="ExternalInput")
    projT_d = nc.dram_tensor("projT", [256, 256], BF16, kind="ExternalInput")
    cbeta_d = nc.dram_tensor("cbeta", [128, 6], F32, kind="ExternalInput")
    cproj_d = nc.dram_tensor("cproj", [128, 2], F32, kind="ExternalInput")
    sel_d = nc.dram_tensor("sel", [2, 128, 32], F32R, kind="ExternalInput")
    selT_d = nc.dram_tensor("selT", [2, 32, 128], F32R, kind="ExternalInput")
    ones_d = nc.dram_tensor("ones", [128, 128], F32R, kind="ExternalInput")
    ones16_d = nc.dram_tensor("ones16", [128, 4], BF16, kind="ExternalInput")
    y_d = nc.dram_tensor("y", [nbatch, 256, 32, 32], F32, kind="ExternalOutput")

    x_ap = x_d.ap().rearrange("b c h w -> b c (h w)")
    y_ap = y_d.ap().rearrange("b c h w -> b c (h w)")

    with tile.TileContext(nc) as tc:
        with tc.tile_pool(name="const", bufs=1) as constp, \
             tc.tile_pool(name="xp", bufs=2) as xp, \
             tc.tile_pool(name="wsp", bufs=2) as wsp, \
             tc.tile_pool(name="qkp", bufs=2) as qkp, \
             tc.tile_pool(name="vtp", bufs=2) as vtp, \
             tc.tile_pool(name="ptp", bufs=32) as ptp, \
             tc.tile_pool(name="sxp", bufs=3) as sxp, \
             tc.tile_pool(name="oap", bufs=2) as oap, \
             tc.tile_pool(name="smallp", bufs=4) as smallp, \
             tc.tile_pool(name="yp", bufs=2) as yp, \
             tc.tile_pool(name="pss", bufs=2, space="PSUM") as pss, \
             tc.tile_pool(name="pso", bufs=2, space="PSUM") as pso, \
             tc.tile_pool(name="psw", bufs=2, space="PSUM") as psw:

            # ---- constants (loaded once) ----
            wqkvT_t = [constp.tile([128, 768], F32R, name=f"wqkvT{t}") for t in range(2)]
            projT_t = [constp.tile([128, 256], BF16, name=f"projT{t}") for t in range(2)]
            cbeta_t = constp.tile([128, 6], F32)
            cproj_t = constp.tile([128, 2], F32)
            sel_t = [constp.tile([128, 32], F32R, name=f"sel{t}") for t in range(2)]
            selT_t = [constp.tile([32, 128], F32R, name=f"selT{t}") for t in range(2)]
            ones_t = constp.tile([128, 128], F32R)
            for t in range(2):
                nc.sync.dma_start(out=wqkvT_t[t][:, :], in_=wqkvT_d.ap()[128*t:128*(t+1), :])
                nc.sync.dma_start(out=projT_t[t][:, :], in_=projT_d.ap()[128*t:128*(t+1), :])
                nc.sync.dma_start(out=sel_t[t][:, :], in_=sel_d.ap()[t, :, :])
                nc.sync.dma_start(out=selT_t[t][:, :], in_=selT_d.ap()[t, :, :])
            nc.sync.dma_start(out=cbeta_t[:, :], in_=cbeta_d.ap()[:, :])
            nc.sync.dma_start(out=cproj_t[:, :], in_=cproj_d.ap()[:, :])
            nc.sync.dma_start(out=ones_t[:, :], in_=ones_d.ap()[:, :])

            def emit_prelude(b, sfx):
                S = {"b": b, "sfx": sfx}
                x_t = [xp.tile([128, 1024], F32R, name=f"x{sfx}t{t}", tag=f"x{t}")
                       for t in range(2)]
                for t in range(2):
                    nc.sync.dma_start(out=x_t[t][:, :], in_=x_ap[b, 128*t:128*(t+1), :])
                S["x"] = x_t

                # GroupNorm stats
                m2mv = []
                for t in range(2):
                    stats = smallp.tile([128, 2, 6], F32, name=f"st{sfx}t{t}", tag="stats")
                    xf = x_t[t][:, :].bitcast(F32).rearrange("p (s n) -> p s n", s=2)
                    nc.vector.bn_stats(out=stats[:, 0, :], in_=xf[:, 0, :])
                    nc.vector.bn_stats(out=stats[:, 1, :], in_=xf[:, 1, :])
                    mv = smallp.tile([128, 2], F32, name=f"mv{sfx}t{t}", tag="mv")
                    nc.vector.bn_aggr(out=mv[:, :], in_=stats[:, :, :])
                    mm = smallp.tile([128, 2], F32R, name=f"mm{sfx}t{t}", tag="mm")
                    nc.vector.tensor_copy(out=mm[:, 0:1], in_=mv[:, 0:1])
                    nc.vector.tensor_scalar(out=mm[:, 1:2], in0=mv[:, 0:1],
                                            scalar1=mv[:, 0:1], scalar2=mv[:, 1:2],
                                            op0=ALU.mult, op1=ALU.add)
                    m2mv.append(mm)
                gstat_ps = psw.tile([32, 2], F32, name=f"gst{sfx}", tag="w")
                for t in range(2):
                    nc.tensor.matmul(gstat_ps[:, :], sel_t[t][:, :].bitcast(F32),
                                     m2mv[t][:, :].bitcast(F32),
                                     start=(t == 0), stop=(t == 1))

                # group mean / rstd (Newton rsqrt; var ~ 1)
                gmu = smallp.tile([32, 2], F32R, name=f"gmu{sfx}", tag="gmu")
                nc.vector.tensor_scalar(out=gmu[:, 0:1], in0=gstat_ps[:, 0:1],
                                        scalar1=0.125, scalar2=None, op0=ALU.mult)
                ta = smallp.tile([32, 4], F32, name=f"ta{sfx}", tag="ta")
                nc.vector.tensor_scalar(out=ta[:, 0:1], in0=gstat_ps[:, 1:2],
                                        scalar1=0.125, scalar2=EPS,
                                        op0=ALU.mult, op1=ALU.add)
                gmuf = gmu[:, 0:1].bitcast(F32)
                nc.vector.tensor_scalar(out=ta[:, 1:2], in0=gmuf, scalar1=gmuf,
                                        scalar2=None, op0=ALU.mult)
                nc.vector.tensor_tensor(out=ta[:, 2:3], in0=ta[:, 0:1], in1=ta[:, 1:2],
                                        op=ALU.subtract)
                nc.vector.tensor_scalar(out=ta[:, 3:4], in0=ta[:, 2:3],
                                        scalar1=-0.5, scalar2=1.5,
                                        op0=ALU.mult, op1=ALU.add)
                for it in range(3):
                    tb = smallp.tile([32, 3], F32, name=f"tb{sfx}i{it}", tag="tb")
                    nc.vector.tensor_tensor(out=tb[:, 0:1], in0=ta[:, 3:4],
                                            in1=ta[:, 3:4], op=ALU.mult)
                    nc.vector.tensor_tensor(out=tb[:, 1:2], in0=tb[:, 0:1],
                                            in1=ta[:, 2:3], op=ALU.mult)
                    nc.vector.tensor_scalar(out=tb[:, 2:3], in0=tb[:, 1:2],
                                            scalar1=-0.5, scalar2=1.5,
                                            op0=ALU.mult, op1=ALU.add)
                    if it < 2:
                        ta2 = smallp.tile([32, 4], F32, name=f"ta{sfx}i{it}", tag="ta")
                        nc.vector.tensor_copy(out=ta2[:, 2:3], in_=ta[:, 2:3])
                        nc.vector.tensor_tensor(out=ta2[:, 3:4], in0=ta[:, 3:4],
                                                in1=tb[:, 2:3], op=ALU.mult)
                        ta = ta2
                    else:
                        nc.vector.tensor_tensor(out=gmu[:, 1:2], in0=ta[:, 3:4],
                                                in1=tb[:, 2:3], op=ALU.mult)

                # broadcast (mu, rstd) to channels; scale W (Pool); biases
                chs = []
                for t in range(2):
                    ch_ps = psw.tile([128, 2], F32, name=f"chp{sfx}t{t}", tag="w")
                    nc.tensor.matmul(ch_ps[:, :], selT_t[t][:, :].bitcast(F32),
                                     gmu[:, :].bitcast(F32), start=True, stop=True)
                    ch = smallp.tile([128, 2], F32R, name=f"chs{sfx}t{t}", tag="chs")
                    nc.vector.tensor_copy(out=ch[:, :], in_=ch_ps[:, :])
                    chs.append(ch)
                wq_s = []
                for t in range(2):
                    ws = wsp.tile([128, 768], F32R, name=f"ws{sfx}t{t}", tag=f"ws{t}")
                    nc.vector.tensor_scalar(out=ws[:, :],
                                            in0=wqkvT_t[t][:, :].bitcast(F32),
                                            scalar1=chs[t][:, 1:2].bitcast(F32),
                                            scalar2=None, op0=ALU.mult)
                    wq_s.append(ws)
                bias_ps = psw.tile([128, 6], F32, name=f"bps{sfx}", tag="w")
                for h in range(6):
                    for t in range(2):
                        nc.tensor.matmul(bias_ps[:, h:h+1],
                                         wq_s[t][:, 128*h:128*(h+1)].bitcast(F32),
                                         chs[t][:, 0:1].bitcast(F32),
                                         start=(t == 0), stop=(t == 1))
                bias_sb = smallp.tile([128, 6], F32R, name=f"bsb{sfx}", tag="bsb")
                nc.vector.tensor_tensor(out=bias_sb[:, :], in0=cbeta_t[:, :],
                                        in1=bias_ps[:, :], op=ALU.subtract)
                # proj-side correction for the v bias
                bias16 = smallp.tile([128, 2], BF16, name=f"b16{sfx}", tag="b16")
                nc.vector.tensor_copy(out=bias16[:, :], in_=bias_sb[:, 4:6].bitcast(F32))
                pb_ps = psw.tile([128, 2], F32, name=f"pbps{sfx}", tag="w")
                for ot in range(2):
                    for t in range(2):
                        nc.tensor.matmul(pb_ps[:, ot:ot+1],
                                         projT_t[t][:, 128*ot:128*(ot+1)],
                                         bias16[:, t:t+1],
                                         start=(t == 0), stop=(t == 1))
                pbias_sb = smallp.tile([128, 2], F32, name=f"pbias{sfx}", tag="pbias")
                nc.vector.tensor_tensor(out=pbias_sb[:, :], in0=cproj_t[:, :],
                                        in1=pb_ps[:, :], op=ALU.add)
                S["pbias"] = pbias_sb

                # qkv production units: emitted lazily so the ACT copies
                # spread across the first slot's score stream instead of one
                # head-of-line burst in the ACT FIFO
                qk_sb = [qkp.tile([128, 1024], BF16, name=f"qk{sfx}o{ot}",
                                  tag=f"qk{ot}") for ot in range(4)]
                vT_sb = [vtp.tile([128, 4, 65], BF16, name=f"vt{sfx}n{nt}",
                                  tag=f"vt{nt}") for nt in range(8)]

                def qk_unit(ot, ih):
                    q_ps = pso.tile([128, 512], F32, name=f"qps{sfx}o{ot}i{ih}",
                                    tag="o")
                    for t in range(2):
                        nc.tensor.matmul(q_ps[:, :],
                                         wq_s[t][:, 128*ot:128*(ot+1)],
                                         x_t[t][:, 512*ih:512*(ih+1)],
                                         start=(t == 0), stop=(t == 1))
                    nc.scalar.activation(
                        out=qk_sb[ot][:, 512*ih:512*(ih+1)],
                        in_=q_ps[:, :], func=AF.Identity,
                        bias=bias_sb[:, ot:ot+1].bitcast(F32), scale=1.0)

                def vt_unit(nt):
                    vt = vT_sb[nt]
                    nc.gpsimd.memset(vt[:, :, 64:65], 1.0)
                    vt_ps = pso.tile([128, 256], F32, name=f"vps{sfx}n{nt}", tag="o")
                    for t in range(2):
                        nc.tensor.matmul(vt_ps[:, :],
                                         x_t[t][:, 128*nt:128*(nt+1)],
                                         wq_s[t][:, 512:768],
                                         start=(t == 0), stop=(t == 1))
                    nc.scalar.activation(
                        out=vt[:, :, 0:64],
                        in_=vt_ps[:, :].rearrange("p (h d) -> p h d", h=4),
                        func=AF.Copy)

                # pair-0 q/k needed before this batch's first score matmul
                for ot in (0, 2):
                    for ih in range(2):
                        qk_unit(ot, ih)
                # pair-1 q/k + all vt deferred into the p0 slot's jt loop
                S["units"] = ([lambda ot=ot, ih=ih: qk_unit(ot, ih)
                               for ot in (1, 3) for ih in range(2)]
                              + [lambda nt=nt: vt_unit(nt) for nt in range(8)])
                S["qk"] = qk_sb
                S["vt"] = vT_sb
                S["pts"] = {h: [] for h in range(4)}
                S["oall"] = [oap.tile([128, 1024], BF16, name=f"oall{sfx}t{t}",
                                      tag=f"oall{t}") for t in range(2)]
                return S

            def emit_scores(S, pair, jt):
                sfx = S["sfx"]
                qk_sb = S["qk"]
                s_ps = {}
                for hh in range(2):
                    h = 2 * pair + hh
                    s_ps[hh] = pss.tile([128, 1024], F32, name=f"sps{sfx}h{h}j{jt}",
                                        tag="s")
                for ih in range(2):
                    for hh in range(2):
                        nc.tensor.matmul(
                            s_ps[hh][:, 512*ih:512*(ih+1)],
                            qk_sb[2 + pair][64*hh:64*hh+64, 128*jt:128*(jt+1)],
                            qk_sb[pair][64*hh:64*hh+64, 512*ih:512*(ih+1)],
                            start=True, stop=True,
                            tile_position=(64*hh, 0))
                for hh in range(2):
                    h = 2 * pair + hh
                    pt = ptp.tile([128, 1024], BF16, name=f"pt{sfx}h{h}j{jt}", tag="pt")
                    if hh == 1 and jt in (0, 2, 3, 5, 6):
                        # Schraudolph on DVE: bf16 bits = round(A*s + B)
                        nc.vector.tensor_scalar(out=pt[:, :].bitcast(I16),
                                                in0=s_ps[hh][:, :],
                                                scalar1=SCH_A, scalar2=SCH_B,
                                                op0=ALU.mult, op1=ALU.add)
                    elif hh == 0 and jt in (1, 4, 6):
                        # hybrid: ACT stages scaled scores to sbuf bf16,
                        # Pool does the Schraudolph affine
                        sx = sxp.tile([128, 1024], BF16, name=f"sx{sfx}h{h}j{jt}",
                                      tag="sx")
                        nc.scalar.activation(out=sx[:, :], in_=s_ps[hh][:, :],
                                             func=AF.Identity, scale=0.125)
                        nc.gpsimd.tensor_scalar(out=pt[:, :].bitcast(I16),
                                                in0=sx[:, :],
                                                scalar1=8.0 * SCH_A, scalar2=SCH_B,
                                                op0=ALU.mult, op1=ALU.add)
                    else:
                        nc.scalar.activation(out=pt[:, :], in_=s_ps[hh][:, :],
                                             func=AF.Exp, scale=0.125)
                    S["pts"][h].append(pt)

            def emit_attnv_norm(S, h, ih):
                sfx = S["sfx"]
                pts = S["pts"][h]
                o_ps = pso.tile([65, 512], F32, name=f"ops{sfx}h{h}i{ih}", tag="o")
                for jt in range(8):
                    nc.tensor.matmul(o_ps[:, :],
                                     S["vt"][jt][:, h, :],
                                     pts[jt][:, 512*ih:512*(ih+1)],
                                     start=(jt == 0), stop=(jt == 7))
                od = smallp.tile([65, 512], F32R, name=f"od{sfx}h{h}i{ih}", tag="od")
                nc.vector.tensor_copy(out=od[:, :], in_=o_ps[:, :])
                r_ps = psw.tile([64, 512], F32, name=f"rps{sfx}h{h}i{ih}", tag="w")
                nc.tensor.matmul(r_ps[:, :], ones_t[64:65, 0:64], od[64:65, :],
                                 start=True, stop=True)
                rr = smallp.tile([64, 512], F32, name=f"rr{sfx}h{h}i{ih}", tag="rr")
                nc.vector.reciprocal_approx_fast(out=rr[:, :], in_=r_ps[:, :])
                nc.gpsimd.tensor_tensor(
                    out=S["oall"][h // 2][64*(h % 2):64*(h % 2)+64,
                                          512*ih:512*(ih+1)],
                    in0=od[0:64, :].bitcast(F32), in1=rr[:, :], op=ALU.mult)

            def emit_proj(S):
                sfx = S["sfx"]
                b = S["b"]
                y_sb = [yp.tile([128, 1024], F32, name=f"y{sfx}t{t}", tag=f"y{t}")
                        for t in range(2)]
                for ot in range(2):
                    for ih in range(2):
                        p_ps = pso.tile([128, 512], F32, name=f"pps{sfx}o{ot}i{ih}",
                                        tag="o")
                        for t in range(2):
                            nc.tensor.matmul(p_ps[:, :],
                                             projT_t[t][:, 128*ot:128*(ot+1)],
                                             S["oall"][t][:, 512*ih:512*(ih+1)],
                                             start=(t == 0), stop=(t == 1))
                        nc.vector.affine_then_add(
                            out=y_sb[ot][:, 512*ih:512*(ih+1)], in0=p_ps[:, :],
                            in1=S["x"][ot][:, 512*ih:512*(ih+1)].bitcast(F32),
                            scale=1.0, bias=S["pbias"][:, ot:ot+1])
                for ot in range(2):
                    nc.sync.dma_start(out=y_ap[b, 128*ot:128*(ot+1), :],
                                      in_=y_sb[ot][:, :])

            unroll = 1
            if loop_reps > 1:
                for u in (4, 2):
                    if loop_reps % u == 0:
                        unroll = u
                        break
            loop_ctx = (tc.For_i(0, loop_reps // unroll, 1,
                                 hint_engines=(mybir.EngineType.PE,))
                        if loop_reps > 1 else contextlib.nullcontext())
            with loop_ctx:
                S_by_b = {}
                prev = None
                for rep in range(unroll):
                    slots = [(b, p) for b in range(nbatch) for p in range(2)]
                    for (b, pair) in slots:
                        if pair == 0:
                            S_by_b[b] = emit_prelude(b, f"r{rep}b{b}")
                        units = S_by_b[b].get("units", []) if pair == 0 else []
                        for jt in range(8):
                            emit_scores(S_by_b[b], pair, jt)
                            for _ in range(2):
                                if units:
                                    units.pop(0)()
                            if prev is not None and jt % 2 == 1:
                                pb, pp = prev
                                blk = jt // 2
                                emit_attnv_norm(S_by_b[pb], 2 * pp + blk // 2, blk % 2)
                                if blk == 3 and pp == 1:
                                    emit_proj(S_by_b[pb])
                        prev = (b, pair)
                        if pair == 0:
                            S_by_b[b]["units"] = units
                pb, pp = prev
                for blk in range(4):
                    emit_attnv_norm(S_by_b[pb], 2 * pp + blk // 2, blk % 2)
                emit_proj(S_by_b[pb])

    nc.compile()
    return nc


def host_constants(gn_w, gn_b, qkv_w, qkv_b, proj_w, proj_b):
    """Fold static parameters into the tensors the kernel expects."""
    import ml_dtypes
    wqkv = qkv_w * gn_w[None, :]             # [768, 256]
    cbeta = qkv_w @ gn_b + qkv_b             # [768]
    sel = np.zeros((2, 128, 32), np.float32)
    selT = np.zeros((2, 32, 128), np.float32)
    for t in range(2):
        for c in range(128):
            g = (128 * t + c) // 8
            sel[t, c, g] = 1.0
            selT[t, g, c] = 1.0
    return {
        "wqkvT": np.ascontiguousarray(wqkv.T).astype(np.float32),   # [256, 768]
        "projT": np.ascontiguousarray(proj_w.T).astype(ml_dtypes.bfloat16),
        "cbeta": np.ascontiguousarray(cbeta.reshape(6, 128).T).astype(np.float32),
        "cproj": np.ascontiguousarray(proj_b.reshape(2, 128).T).astype(np.float32),
        "sel": sel, "selT": selT,
        "ones": np.ones((128, 128), np.float32),
        "ones16": np.ones((128, 4), ml_dtypes.bfloat16),
    }


_CACHE = {}


def kernel(x, gn_w, gn_b, qkv_w, qkv_b, proj_w, proj_b):
    from concourse.bass_utils import run_bass_kernel_spmd

    x = np.asarray(x, dtype=np.float32)
    consts = host_constants(np.asarray(gn_w, np.float32), np.asarray(gn_b, np.float32),
                            np.asarray(qkv_w, np.float32), np.asarray(qkv_b, np.float32),
                            np.asarray(proj_w, np.float32), np.asarray(proj_b, np.float32))
    if "nc" not in _CACHE:
        _CACHE["nc"] = build()
    nc = _CACHE["nc"]
    n_cores = 8
    bpc = x.shape[0] // n_cores
    in_maps = [{"x": np.ascontiguousarray(x[bpc*i:bpc*(i+1)]), **consts}
               for i in range(n_cores)]
    res = run_bass_kernel_spmd(nc, in_maps, core_ids=list(range(n_cores)))
    return np.concatenate([res.results[i]["y"] for i in range(n_cores)],
                          axis=0).astype(np.float32)



# revision 2
# speedup vs baseline: 1.0207x; 1.0207x over previous
"""V2.1: engine-rebalanced AttentionBlock kernel + swapped attn@v.

Changes vs baseline:
- softmax exp split: ~2/3 of tiles exact exp on ACT, ~1/3 Schraudolph
  bit-trick (f32->int16 affine, bitcast bf16) on DVE
- qk bias-add + vt psum->sbuf copies moved to ACT (activation Copy)
- qkv weight scaling + attn-normalize mult moved to Pool (gpsimd)
- vt ones-column init via Pool memset instead of DMA
"""
import contextlib
import numpy as np
import concourse.bacc as bacc
import concourse.tile as tile
from concourse import mybir, masks

F32 = mybir.dt.float32
F32R = mybir.dt.float32r
BF16 = mybir.dt.bfloat16
I16 = mybir.dt.int16
AF = mybir.ActivationFunctionType
ALU = mybir.AluOpType

B_PER_CORE = 2
EPS = 1e-5

# Schraudolph exp: exp(0.125*s) ~= bf16_bits(round(A*s + B))
LOG2E = 1.4426950408889634
SCH_A = 0.125 * 128.0 * LOG2E
SCH_B = 127.0 * 128.0 - 11.0


def build(nbatch=B_PER_CORE, loop_reps=1):
    nc = bacc.Bacc("TRN2", target_bir_lowering=False, debug=False)

    x_d = nc.dram_tensor("x", [nbatch, 256, 32, 32], F32R, kind="ExternalInput")
    wqkvT_d = nc.dram_tensor("wqkvT", [256, 768], F32R, kind="ExternalInput")
    projT_d = nc.dram_tensor("projT", [256, 256], BF16, kind="ExternalInput")
    cbeta_d = nc.dram_tensor("cbeta", [128, 6], F32, kind="ExternalInput")
    cproj_d = nc.dram_tensor("cproj", [128, 2], F32, kind="ExternalInput")
    sel_d = nc.dram_tensor("sel", [2, 128, 32], F32R, kind="ExternalInput")
    selT_d = nc.dram_tensor("selT", [2, 32, 128], F32R, kind="ExternalInput")
    y_d = nc.dram_tensor("y", [nbatch, 256, 32, 32], F32, kind="ExternalOutput")

    x_ap = x_d.ap().rearrange("b c h w -> b c (h w)")
    y_ap = y_d.ap().rearrange("b c h w -> b c (h w)")

    with tile.TileContext(nc) as tc:
        with tc.tile_pool(name="const", bufs=1) as constp, \
             tc.tile_pool(name="xp", bufs=2) as xp, \
             tc.tile_pool(name="wsp", bufs=2) as wsp, \
             tc.tile_pool(name="qkp", bufs=2) as qkp, \
             tc.tile_pool(name="vtp", bufs=2) as vtp, \
             tc.tile_pool(name="ptp", bufs=32) as ptp, \
             tc.tile_pool(name="sxp", bufs=3) as sxp, \
             tc.tile_pool(name="oap", bufs=2) as oap, \
             tc.tile_pool(name="smallp", bufs=4) as smallp, \
             tc.tile_pool(name="yp", bufs=2) as yp, \
             tc.tile_pool(name="pss", bufs=2, space="PSUM") as pss, \
             tc.tile_pool(name="pso", bufs=3, space="PSUM") as pso, \
             tc.tile_pool(name="pst", bufs=1, space="PSUM") as pst:

            # ---- constants (loaded once) ----
            wqkvT_t = [constp.tile([128, 768], F32R, name=f"wqkvT{t}") for t in range(2)]
            projT_t = [constp.tile([128, 256], BF16, name=f"projT{t}") for t in range(2)]
            cbeta_t = constp.tile([128, 6], F32)
            cproj_t = constp.tile([128, 2], F32)
            sel_t = [constp.tile([128, 32], F32R, name=f"sel{t}") for t in range(2)]
            selT_t = [constp.tile([32, 128], F32R, name=f"selT{t}") for t in range(2)]
            ident_t = constp.tile([128, 128], BF16)
            for t in range(2):
                nc.sync.dma_start(out=wqkvT_t[t][:, :], in_=wqkvT_d.ap()[128*t:128*(t+1), :])
                nc.sync.dma_start(out=projT_t[t][:, :], in_=projT_d.ap()[128*t:128*(t+1), :])
                nc.sync.dma_start(out=sel_t[t][:, :], in_=sel_d.ap()[t, :, :])
                nc.sync.dma_start(out=selT_t[t][:, :], in_=selT_d.ap()[t, :, :])
            nc.sync.dma_start(out=cbeta_t[:, :], in_=cbeta_d.ap()[:, :])
            nc.sync.dma_start(out=cproj_t[:, :], in_=cproj_d.ap()[:, :])
            masks.make_identity(nc, ident_t[:, :])

            def emit_prelude(b, sfx):
                S = {"b": b, "sfx": sfx}
                x_t = [xp.tile([128, 1024], F32R, name=f"x{sfx}t{t}", tag=f"x{t}")
                       for t in range(2)]
                for t in range(2):
                    nc.sync.dma_start(out=x_t[t][:, :], in_=x_ap[b, 128*t:128*(t+1), :])
                S["x"] = x_t

                # GroupNorm stats
                m2mv = []
                for t in range(2):
                    stats = smallp.tile([128, 2, 6], F32, name=f"st{sfx}t{t}", tag="stats")
                    xf = x_t[t][:, :].bitcast(F32).rearrange("p (s n) -> p s n", s=2)
                    nc.vector.bn_stats(out=stats[:, 0, :], in_=xf[:, 0, :])
                    nc.vector.bn_stats(out=stats[:, 1, :], in_=xf[:, 1, :])
                    mv = smallp.tile([128, 2], F32, name=f"mv{sfx}t{t}", tag="mv")
                    nc.vector.bn_aggr(out=mv[:, :], in_=stats[:, :, :])
                    mm = smallp.tile([128, 2], F32R, name=f"mm{sfx}t{t}", tag="mm")
                    nc.vector.tensor_copy(out=mm[:, 0:1], in_=mv[:, 0:1])
                    nc.vector.tensor_scalar(out=mm[:, 1:2], in0=mv[:, 0:1],
                                            scalar1=mv[:, 0:1], scalar2=mv[:, 1:2],
                                            op0=ALU.mult, op1=ALU.add)
                    m2mv.append(mm)
                gstat_ps = pso.tile([32, 512], F32, name=f"gst{sfx}", tag="o")
                for t in range(2):
                    nc.tensor.matmul(gstat_ps[:, 0:2], sel_t[t][:, :].bitcast(F32),
                                     m2mv[t][:, :].bitcast(F32),
                                     start=(t == 0), stop=(t == 1))

                # group mean / rstd (Newton rsqrt; var ~ 1)
                gmu = smallp.tile([32, 2], F32R, name=f"gmu{sfx}", tag="gmu")
                nc.vector.tensor_scalar(out=gmu[:, 0:1], in0=gstat_ps[:, 0:1],
                                        scalar1=0.125, scalar2=None, op0=ALU.mult)
                ta = smallp.tile([32, 4], F32, name=f"ta{sfx}", tag="ta")
                nc.vector.tensor_scalar(out=ta[:, 0:1], in0=gstat_ps[:, 1:2],
                                        scalar1=0.125, scalar2=EPS,
                                        op0=ALU.mult, op1=ALU.add)
                gmuf = gmu[:, 0:1].bitcast(F32)
                nc.vector.tensor_scalar(out=ta[:, 1:2], in0=gmuf, scalar1=gmuf,
                                        scalar2=None, op0=ALU.mult)
                nc.vector.tensor_tensor(out=ta[:, 2:3], in0=ta[:, 0:1], in1=ta[:, 1:2],
                                        op=ALU.subtract)
                nc.vector.tensor_scalar(out=ta[:, 3:4], in0=ta[:, 2:3],
                                        scalar1=-0.5, scalar2=1.5,
                                        op0=ALU.mult, op1=ALU.add)
                for it in range(3):
                    tb = smallp.tile([32, 3], F32, name=f"tb{sfx}i{it}", tag="tb")
                    nc.vector.tensor_tensor(out=tb[:, 0:1], in0=ta[:, 3:4],
                                            in1=ta[:, 3:4], op=ALU.mult)
                    nc.vector.tensor_tensor(out=tb[:, 1:2], in0=tb[:, 0:1],
                                            in1=ta[:, 2:3], op=ALU.mult)
                    nc.vector.tensor_scalar(out=tb[:, 2:3], in0=tb[:, 1:2],
                                            scalar1=-0.5, scalar2=1.5,
                                            op0=ALU.mult, op1=ALU.add)
                    if it < 2:
                        ta2 = smallp.tile([32, 4], F32, name=f"ta{sfx}i{it}", tag="ta")
                        nc.vector.tensor_copy(out=ta2[:, 2:3], in_=ta[:, 2:3])
                        nc.vector.tensor_tensor(out=ta2[:, 3:4], in0=ta[:, 3:4],
                                                in1=tb[:, 2:3], op=ALU.mult)
                        ta = ta2
                    else:
                        nc.vector.tensor_tensor(out=gmu[:, 1:2], in0=ta[:, 3:4],
                                                in1=tb[:, 2:3], op=ALU.mult)

                # broadcast (mu, rstd) to channels; scale W (Pool); biases
                chs = []
                for t in range(2):
                    ch_ps = pso.tile([128, 512], F32, name=f"chp{sfx}t{t}", tag="o")
                    nc.tensor.matmul(ch_ps[:, 0:2], selT_t[t][:, :].bitcast(F32),
                                     gmu[:, :].bitcast(F32), start=True, stop=True)
                    ch = smallp.tile([128, 2], F32R, name=f"chs{sfx}t{t}", tag="chs")
                    nc.vector.tensor_copy(out=ch[:, :], in_=ch_ps[:, 0:2])
                    chs.append(ch)
                wq_s = []
                for t in range(2):
                    ws = wsp.tile([128, 768], F32R, name=f"ws{sfx}t{t}", tag=f"ws{t}")
                    nc.vector.tensor_scalar(out=ws[:, :],
                                            in0=wqkvT_t[t][:, :].bitcast(F32),
                                            scalar1=chs[t][:, 1:2].bitcast(F32),
                                            scalar2=None, op0=ALU.mult)
                    wq_s.append(ws)
                bias_ps = pso.tile([128, 512], F32, name=f"bps{sfx}", tag="o")
                for h in range(6):
                    for t in range(2):
                        nc.tensor.matmul(bias_ps[:, h:h+1],
                                         wq_s[t][:, 128*h:128*(h+1)].bitcast(F32),
                                         chs[t][:, 0:1].bitcast(F32),
                                         start=(t == 0), stop=(t == 1),
                                         skip_group_check=(h > 0 and t == 0))
                bias_sb = smallp.tile([128, 6], F32R, name=f"bsb{sfx}", tag="bsb")
                nc.vector.tensor_tensor(out=bias_sb[:, :], in0=cbeta_t[:, :],
                                        in1=bias_ps[:, 0:6], op=ALU.subtract)
                # proj-side correction for the v bias
                bias16 = smallp.tile([128, 2], BF16, name=f"b16{sfx}", tag="b16")
                nc.vector.tensor_copy(out=bias16[:, :], in_=bias_sb[:, 4:6].bitcast(F32))
                pb_ps = pso.tile([128, 512], F32, name=f"pbps{sfx}", tag="o")
                for ot in range(2):
                    for t in range(2):
                        nc.tensor.matmul(pb_ps[:, ot:ot+1],
                                         projT_t[t][:, 128*ot:128*(ot+1)],
                                         bias16[:, t:t+1],
                                         start=(t == 0), stop=(t == 1),
                                         skip_group_check=(ot == 1 and t == 0))
                pbias_sb = smallp.tile([128, 2], F32, name=f"pbias{sfx}", tag="pbias")
                nc.vector.tensor_tensor(out=pbias_sb[:, :], in0=cproj_t[:, :],
                                        in1=pb_ps[:, 0:2], op=ALU.add)
                S["pbias"] = pbias_sb

                # qkv production units: emitted lazily so the ACT copies
                # spread across the first slot's score stream instead of one
                # head-of-line burst in the ACT FIFO
                qk_sb = [qkp.tile([128, 1024], BF16, name=f"qk{sfx}o{ot}",
                                  tag=f"qk{ot}") for ot in range(4)]
                vT_sb = [vtp.tile([128, 4, 65], BF16, name=f"vt{sfx}n{nt}",
                                  tag=f"vt{nt}") for nt in range(8)]

                def qk_unit(ot, ih):
                    q_ps = pso.tile([128, 512], F32, name=f"qps{sfx}o{ot}i{ih}",
                                    tag="o")
                    for t in range(2):
                        nc.tensor.matmul(q_ps[:, :],
                                         wq_s[t][:, 128*ot:128*(ot+1)],
                                         x_t[t][:, 512*ih:512*(ih+1)],
                                         start=(t == 0), stop=(t == 1))
                    nc.scalar.activation(
                        out=qk_sb[ot][:, 512*ih:512*(ih+1)],
                        in_=q_ps[:, :], func=AF.Identity,
                        bias=bias_sb[:, ot:ot+1].bitcast(F32), scale=1.0)

                def vt_unit(nt):
                    vt = vT_sb[nt]
                    nc.gpsimd.memset(vt[:, :, 64:65], 1.0)
                    vt_ps = pso.tile([128, 256], F32, name=f"vps{sfx}n{nt}", tag="o")
                    for t in range(2):
                        nc.tensor.matmul(vt_ps[:, :],
                                         x_t[t][:, 128*nt:128*(nt+1)],
                                         wq_s[t][:, 512:768],
                                         start=(t == 0), stop=(t == 1))
                    nc.scalar.activation(
                        out=vt[:, :, 0:64],
                        in_=vt_ps[:, :].rearrange("p (h d) -> p h d", h=4),
                        func=AF.Copy)

                # pair-0 q/k needed before this batch's first score matmul
                for ot in (0, 2):
                    for ih in range(2):
                        qk_unit(ot, ih)
                # pair-1 q/k + all vt deferred into the p0 slot's jt loop
                S["units"] = ([lambda ot=ot, ih=ih: qk_unit(ot, ih)
                               for ot in (1, 3) for ih in range(2)]
                              + [lambda nt=nt: vt_unit(nt) for nt in range(8)])
                S["qk"] = qk_sb
                S["vt"] = vT_sb
                S["pts"] = {h: {} for h in range(4)}
                S["oall"] = [oap.tile([128, 1024], BF16, name=f"oall{sfx}t{t}",
                                      tag=f"oall{t}") for t in range(2)]
                return S

            def emit_scores(S, pair, jt):
                sfx = S["sfx"]
                qk_sb = S["qk"]
                s_ps = {}
                for hh in range(2):
                    h = 2 * pair + hh
                    s_ps[hh] = pss.tile([128, 1024], F32, name=f"sps{sfx}h{h}j{jt}",
                                        tag="s")
                for ih in range(2):
                    for hh in range(2):
                        nc.tensor.matmul(
                            s_ps[hh][:, 512*ih:512*(ih+1)],
                            qk_sb[2 + pair][64*hh:64*hh+64, 128*jt:128*(jt+1)],
                            qk_sb[pair][64*hh:64*hh+64, 512*ih:512*(ih+1)],
                            start=True, stop=True,
                            tile_position=(64*hh, 0))
                for hh in range(2):
                    h = 2 * pair + hh
                    pt = ptp.tile([128, 1024], BF16, name=f"pt{sfx}h{h}j{jt}", tag="pt")
                    if hh == 1 and jt in (0, 2, 3, 5, 6):
                        # Schraudolph on DVE: bf16 bits = round(A*s + B)
                        nc.vector.tensor_scalar(out=pt[:, :].bitcast(I16),
                                                in0=s_ps[hh][:, :],
                                                scalar1=SCH_A, scalar2=SCH_B,
                                                op0=ALU.mult, op1=ALU.add)
                    elif hh == 0 and jt in (1, 4, 6):
                        # hybrid: ACT stages scaled scores to sbuf bf16,
                        # Pool does the Schraudolph affine
                        sx = sxp.tile([128, 1024], BF16, name=f"sx{sfx}h{h}j{jt}",
                                      tag="sx")
                        nc.scalar.activation(out=sx[:, :], in_=s_ps[hh][:, :],
                                             func=AF.Identity, scale=0.125)
                        nc.gpsimd.tensor_scalar(out=pt[:, :].bitcast(I16),
                                                in0=sx[:, :],
                                                scalar1=8.0 * SCH_A, scalar2=SCH_B,
                                                op0=ALU.mult, op1=ALU.add)
                    else:
                        nc.scalar.activation(out=pt[:, :], in_=s_ps[hh][:, :],
                                             func=AF.Exp, scale=0.125)
                    S["pts"][h][jt] = pt

            def emit_attnv_norm(S, pair, u):
                sfx = S["sfx"]
                pts = S["pts"]
                if u % 2 == 0:
                    S[f"tr{pair}g{u//2}"] = pst.tile(
                        [128, 512], BF16, name=f"tr{sfx}p{pair}g{u//2}", tag="tr")
                tr_ps = S[f"tr{pair}g{u//2}"]
                o_ps = pso.tile([128, 512], F32, name=f"ops{sfx}p{pair}u{u}", tag="o")
                for g in range(4):          # g = 2*(it-2u) + hh
                    it = 2 * u + g // 2
                    hh = g % 2
                    h = 2 * pair + hh
                    for jt in range(8):
                        nc.tensor.matmul(
                            o_ps[:, 66*g:66*g+65],
                            pts[h][jt][:, 128*it:128*(it+1)],
                            S["vt"][jt][:, h, :],
                            start=(g == 0 and jt == 0), stop=(jt == 7),
                            skip_group_check=(g > 0 and jt == 0))
                rec = smallp.tile([128, 4], F32, name=f"rec{sfx}p{pair}u{u}",
                                  tag="rec")
                nc.vector.reciprocal_approx_fast(
                    out=rec[:, :].rearrange("p (g c) -> p g c", c=1),
                    in_=o_ps[:, 0:264].rearrange("p (g c) -> p g c", g=4)[:, :, 64:65])
                for k in range(2):          # k = it - 2u
                    it = 2 * u + k
                    ut = sxp.tile([128, 128], BF16, name=f"u{sfx}p{pair}i{it}",
                                  tag="u")
                    for hh in range(2):
                        g = 2 * k + hh
                        if hh == 0:
                            nc.vector.tensor_scalar(
                                out=ut[:, 0:64], in0=o_ps[:, 66*g:66*g+64],
                                scalar1=rec[:, g:g+1], scalar2=None, op0=ALU.mult)
                        else:
                            nc.scalar.activation(
                                out=ut[:, 64:128], in_=o_ps[:, 66*g:66*g+64],
                                func=AF.Copy, scale=rec[:, g:g+1])
                    nc.tensor.transpose(
                        out=tr_ps[:, 128*(it % 4):128*(it % 4)+128],
                        in_=ut[:, :], identity=ident_t[:, :])
                if u % 2 == 1:
                    nc.scalar.activation(
                        out=S["oall"][pair][:, 512*(u//2):512*(u//2)+512],
                        in_=tr_ps[:, :], func=AF.Copy)

            def emit_proj(S):
                sfx = S["sfx"]
                b = S["b"]
                y_sb = [yp.tile([128, 1024], F32, name=f"y{sfx}t{t}", tag=f"y{t}")
                        for t in range(2)]
                for ot in range(2):
                    for ih in range(2):
                        p_ps = pso.tile([128, 512], F32, name=f"pps{sfx}o{ot}i{ih}",
                                        tag="o")
                        for t in range(2):
                            nc.tensor.matmul(p_ps[:, :],
                                             projT_t[t][:, 128*ot:128*(ot+1)],
                                             S["oall"][t][:, 512*ih:512*(ih+1)],
                                             start=(t == 0), stop=(t == 1))
                        nc.vector.affine_then_add(
                            out=y_sb[ot][:, 512*ih:512*(ih+1)], in0=p_ps[:, :],
                            in1=S["x"][ot][:, 512*ih:512*(ih+1)].bitcast(F32),
                            scale=1.0, bias=S["pbias"][:, ot:ot+1])
                for ot in range(2):
                    nc.sync.dma_start(out=y_ap[b, 128*ot:128*(ot+1), :],
                                      in_=y_sb[ot][:, :])

            unroll = 1
            if loop_reps > 1:
                for u in (4, 2):
                    if loop_reps % u == 0:
                        unroll = u
                        break
            loop_ctx = (tc.For_i(0, loop_reps // unroll, 1,
                                 hint_engines=(mybir.EngineType.PE,))
                        if loop_reps > 1 else contextlib.nullcontext())
            with loop_ctx:
                S_by_b = {}
                prev = None
                for rep in range(unroll):
                    slots = [(b, p) for b in range(nbatch) for p in range(2)]
                    for (b, pair) in slots:
                        if pair == 0:
                            S_by_b[b] = emit_prelude(b, f"r{rep}b{b}")
                        units = S_by_b[b].get("units", []) if pair == 0 else []
                        for jt in range(8):
                            emit_scores(S_by_b[b], pair, jt)
                            for _ in range(2):
                                if units:
                                    units.pop(0)()
                            if prev is not None and jt % 2 == 1:
                                pb, pp = prev
                                blk = jt // 2
                                emit_attnv_norm(S_by_b[pb], pp, blk)
                                if blk == 3 and pp == 1:
                                    emit_proj(S_by_b[pb])
                        prev = (b, pair)
                        if pair == 0:
                            S_by_b[b]["units"] = units
                pb, pp = prev
                for blk in range(4):
                    emit_attnv_norm(S_by_b[pb], pp, blk)
                emit_proj(S_by_b[pb])

    nc.compile()
    return nc


def host_constants(gn_w, gn_b, qkv_w, qkv_b, proj_w, proj_b):
    """Fold static parameters into the tensors the kernel expects."""
    import ml_dtypes
    wqkv = qkv_w * gn_w[None, :]             # [768, 256]
    cbeta = qkv_w @ gn_b + qkv_b             # [768]
    sel = np.zeros((2, 128, 32), np.float32)
    selT = np.zeros((2, 32, 128), np.float32)
    for t in range(2):
        for c in range(128):
            g = (128 * t + c) // 8
            sel[t, c, g] = 1.0
            selT[t, g, c] = 1.0
    return {
        "wqkvT": np.ascontiguousarray(wqkv.T).astype(np.float32),   # [256, 768]
        "projT": np.ascontiguousarray(proj_w.T).astype(ml_dtypes.bfloat16),
        "cbeta": np.ascontiguousarray(cbeta.reshape(6, 128).T).astype(np.float32),
        "cproj": np.ascontiguousarray(proj_b.reshape(2, 128).T).astype(np.float32),
        "sel": sel, "selT": selT,
    }


_CACHE = {}


def kernel(x, gn_w, gn_b, qkv_w, qkv_b, proj_w, proj_b):
    from concourse.bass_utils import run_bass_kernel_spmd

    x = np.asarray(x, dtype=np.float32)
    consts = host_constants(np.asarray(gn_w, np.float32), np.asarray(gn_b, np.float32),
                            np.asarray(qkv_w, np.float32), np.asarray(qkv_b, np.float32),
                            np.asarray(proj_w, np.float32), np.asarray(proj_b, np.float32))
    if "nc" not in _CACHE:
        _CACHE["nc"] = build()
    nc = _CACHE["nc"]
    n_cores = 8
    bpc = x.shape[0] // n_cores
    in_maps = [{"x": np.ascontiguousarray(x[bpc*i:bpc*(i+1)]), **consts}
               for i in range(n_cores)]
    res = run_bass_kernel_spmd(nc, in_maps, core_ids=list(range(n_cores)))
    return np.concatenate([res.results[i]["y"] for i in range(n_cores)],
                          axis=0).astype(np.float32)



# revision 3
# speedup vs baseline: 1.0566x; 1.0352x over previous
"""V2.2: engine-rebalanced AttentionBlock kernel + swapped attn@v + fast rsqrt.

Changes vs baseline:
- softmax exp split: ~2/3 of tiles exact exp on ACT, ~1/3 Schraudolph
  bit-trick (f32->int16 affine, bitcast bf16) on DVE
- qk bias-add + vt psum->sbuf copies moved to ACT (activation Copy)
- qkv weight scaling + attn-normalize mult moved to Pool (gpsimd)
- vt ones-column init via Pool memset instead of DMA
"""
import contextlib
import numpy as np
import concourse.bacc as bacc
import concourse.tile as tile
from concourse import mybir, masks

F32 = mybir.dt.float32
F32R = mybir.dt.float32r
BF16 = mybir.dt.bfloat16
I16 = mybir.dt.int16
AF = mybir.ActivationFunctionType
ALU = mybir.AluOpType

B_PER_CORE = 2
EPS = 1e-5

# Schraudolph exp: exp(0.125*s) ~= bf16_bits(round(A*s + B))
LOG2E = 1.4426950408889634
SCH_A = 0.125 * 128.0 * LOG2E
SCH_B = 127.0 * 128.0 - 11.0


def build(nbatch=B_PER_CORE, loop_reps=1):
    nc = bacc.Bacc("TRN2", target_bir_lowering=False, debug=False)

    x_d = nc.dram_tensor("x", [nbatch, 256, 32, 32], F32R, kind="ExternalInput")
    wqkvT_d = nc.dram_tensor("wqkvT", [256, 768], F32R, kind="ExternalInput")
    projT_d = nc.dram_tensor("projT", [256, 256], BF16, kind="ExternalInput")
    cbeta_d = nc.dram_tensor("cbeta", [128, 6], F32, kind="ExternalInput")
    cproj_d = nc.dram_tensor("cproj", [128, 2], F32, kind="ExternalInput")
    sel_d = nc.dram_tensor("sel", [2, 128, 32], F32R, kind="ExternalInput")
    selT_d = nc.dram_tensor("selT", [2, 32, 128], F32R, kind="ExternalInput")
    y_d = nc.dram_tensor("y", [nbatch, 256, 32, 32], F32, kind="ExternalOutput")

    x_ap = x_d.ap().rearrange("b c h w -> b c (h w)")
    y_ap = y_d.ap().rearrange("b c h w -> b c (h w)")

    with tile.TileContext(nc) as tc:
        with tc.tile_pool(name="const", bufs=1) as constp, \
             tc.tile_pool(name="xp", bufs=2) as xp, \
             tc.tile_pool(name="wsp", bufs=2) as wsp, \
             tc.tile_pool(name="qkp", bufs=2) as qkp, \
             tc.tile_pool(name="vtp", bufs=2) as vtp, \
             tc.tile_pool(name="ptp", bufs=32) as ptp, \
             tc.tile_pool(name="sxp", bufs=3) as sxp, \
             tc.tile_pool(name="oap", bufs=2) as oap, \
             tc.tile_pool(name="smallp", bufs=4) as smallp, \
             tc.tile_pool(name="yp", bufs=2) as yp, \
             tc.tile_pool(name="pss", bufs=2, space="PSUM") as pss, \
             tc.tile_pool(name="pso", bufs=3, space="PSUM") as pso, \
             tc.tile_pool(name="pst", bufs=1, space="PSUM") as pst:

            # ---- constants (loaded once) ----
            wqkvT_t = [constp.tile([128, 768], F32R, name=f"wqkvT{t}") for t in range(2)]
            projT_t = [constp.tile([128, 256], BF16, name=f"projT{t}") for t in range(2)]
            cbeta_t = constp.tile([128, 6], F32)
            cproj_t = constp.tile([128, 2], F32)
            sel_t = [constp.tile([128, 32], F32R, name=f"sel{t}") for t in range(2)]
            selT_t = [constp.tile([32, 128], F32R, name=f"selT{t}") for t in range(2)]
            ident_t = constp.tile([128, 128], BF16)
            for t in range(2):
                nc.sync.dma_start(out=wqkvT_t[t][:, :], in_=wqkvT_d.ap()[128*t:128*(t+1), :])
                nc.sync.dma_start(out=projT_t[t][:, :], in_=projT_d.ap()[128*t:128*(t+1), :])
                nc.sync.dma_start(out=sel_t[t][:, :], in_=sel_d.ap()[t, :, :])
                nc.sync.dma_start(out=selT_t[t][:, :], in_=selT_d.ap()[t, :, :])
            nc.sync.dma_start(out=cbeta_t[:, :], in_=cbeta_d.ap()[:, :])
            nc.sync.dma_start(out=cproj_t[:, :], in_=cproj_d.ap()[:, :])
            masks.make_identity(nc, ident_t[:, :])

            def emit_prelude(b, sfx):
                S = {"b": b, "sfx": sfx}
                x_t = [xp.tile([128, 1024], F32R, name=f"x{sfx}t{t}", tag=f"x{t}")
                       for t in range(2)]
                for t in range(2):
                    nc.sync.dma_start(out=x_t[t][:, :], in_=x_ap[b, 128*t:128*(t+1), :])
                S["x"] = x_t

                # GroupNorm stats
                m2mv = []
                for t in range(2):
                    stats = smallp.tile([128, 2, 6], F32, name=f"st{sfx}t{t}", tag="stats")
                    xf = x_t[t][:, :].bitcast(F32).rearrange("p (s n) -> p s n", s=2)
                    nc.vector.bn_stats(out=stats[:, 0, :], in_=xf[:, 0, :])
                    nc.vector.bn_stats(out=stats[:, 1, :], in_=xf[:, 1, :])
                    mv = smallp.tile([128, 2], F32, name=f"mv{sfx}t{t}", tag="mv")
                    nc.vector.bn_aggr(out=mv[:, :], in_=stats[:, :, :])
                    mm = smallp.tile([128, 2], F32R, name=f"mm{sfx}t{t}", tag="mm")
                    nc.vector.tensor_copy(out=mm[:, 0:1], in_=mv[:, 0:1])
                    nc.vector.tensor_scalar(out=mm[:, 1:2], in0=mv[:, 0:1],
                                            scalar1=mv[:, 0:1], scalar2=mv[:, 1:2],
                                            op0=ALU.mult, op1=ALU.add)
                    m2mv.append(mm)
                gstat_ps = pso.tile([32, 512], F32, name=f"gst{sfx}", tag="o")
                for t in range(2):
                    nc.tensor.matmul(gstat_ps[:, 0:2], sel_t[t][:, :].bitcast(F32),
                                     m2mv[t][:, :].bitcast(F32),
                                     start=(t == 0), stop=(t == 1))

                # group mean / rstd (Newton rsqrt; var ~ 1)
                gmu = smallp.tile([32, 2], F32R, name=f"gmu{sfx}", tag="gmu")
                nc.vector.tensor_scalar(out=gmu[:, 0:1], in0=gstat_ps[:, 0:1],
                                        scalar1=0.125, scalar2=None, op0=ALU.mult)
                ta = smallp.tile([32, 4], F32, name=f"ta{sfx}", tag="ta")
                nc.vector.tensor_scalar(out=ta[:, 0:1], in0=gstat_ps[:, 1:2],
                                        scalar1=0.125, scalar2=EPS,
                                        op0=ALU.mult, op1=ALU.add)
                gmuf = gmu[:, 0:1].bitcast(F32)
                nc.vector.tensor_scalar(out=ta[:, 1:2], in0=gmuf, scalar1=gmuf,
                                        scalar2=None, op0=ALU.mult)
                nc.vector.tensor_tensor(out=ta[:, 2:3], in0=ta[:, 0:1], in1=ta[:, 1:2],
                                        op=ALU.subtract)
                nc.scalar.activation(out=ta[:, 3:4], in_=ta[:, 2:3], func=AF.Sqrt)
                nc.vector.reciprocal_approx_fast(out=gmu[:, 1:2].bitcast(F32),
                                                 in_=ta[:, 3:4])

                # broadcast (mu, rstd) to channels; scale W (Pool); biases
                chs = []
                for t in range(2):
                    ch_ps = pso.tile([128, 512], F32, name=f"chp{sfx}t{t}", tag="o")
                    nc.tensor.matmul(ch_ps[:, 0:2], selT_t[t][:, :].bitcast(F32),
                                     gmu[:, :].bitcast(F32), start=True, stop=True)
                    ch = smallp.tile([128, 2], F32R, name=f"chs{sfx}t{t}", tag="chs")
                    nc.vector.tensor_copy(out=ch[:, :], in_=ch_ps[:, 0:2])
                    chs.append(ch)
                wq_s = []
                for t in range(2):
                    ws = wsp.tile([128, 768], F32R, name=f"ws{sfx}t{t}", tag=f"ws{t}")
                    nc.vector.tensor_scalar(out=ws[:, :],
                                            in0=wqkvT_t[t][:, :].bitcast(F32),
                                            scalar1=chs[t][:, 1:2].bitcast(F32),
                                            scalar2=None, op0=ALU.mult)
                    wq_s.append(ws)
                bias_ps = pso.tile([128, 512], F32, name=f"bps{sfx}", tag="o")
                for h in range(6):
                    for t in range(2):
                        nc.tensor.matmul(bias_ps[:, h:h+1],
                                         wq_s[t][:, 128*h:128*(h+1)].bitcast(F32),
                                         chs[t][:, 0:1].bitcast(F32),
                                         start=(t == 0), stop=(t == 1),
                                         skip_group_check=(h > 0 and t == 0))
                bias_sb = smallp.tile([128, 6], F32R, name=f"bsb{sfx}", tag="bsb")
                nc.vector.tensor_tensor(out=bias_sb[:, :], in0=cbeta_t[:, :],
                                        in1=bias_ps[:, 0:6], op=ALU.subtract)
                # proj-side correction for the v bias
                bias16 = smallp.tile([128, 2], BF16, name=f"b16{sfx}", tag="b16")
                nc.vector.tensor_copy(out=bias16[:, :], in_=bias_sb[:, 4:6].bitcast(F32))
                pb_ps = pso.tile([128, 512], F32, name=f"pbps{sfx}", tag="o")
                for ot in range(2):
                    for t in range(2):
                        nc.tensor.matmul(pb_ps[:, ot:ot+1],
                                         projT_t[t][:, 128*ot:128*(ot+1)],
                                         bias16[:, t:t+1],
                                         start=(t == 0), stop=(t == 1),
                                         skip_group_check=(ot == 1 and t == 0))
                pbias_sb = smallp.tile([128, 2], F32, name=f"pbias{sfx}", tag="pbias")
                nc.vector.tensor_tensor(out=pbias_sb[:, :], in0=cproj_t[:, :],
                                        in1=pb_ps[:, 0:2], op=ALU.add)
                S["pbias"] = pbias_sb

                # qkv production units: emitted lazily so the ACT copies
                # spread across the first slot's score stream instead of one
                # head-of-line burst in the ACT FIFO
                qk_sb = [qkp.tile([128, 1024], BF16, name=f"qk{sfx}o{ot}",
                                  tag=f"qk{ot}") for ot in range(4)]
                vT_sb = [vtp.tile([128, 4, 65], BF16, name=f"vt{sfx}n{nt}",
                                  tag=f"vt{nt}") for nt in range(8)]

                def qk_unit(ot, ih):
                    q_ps = pso.tile([128, 512], F32, name=f"qps{sfx}o{ot}i{ih}",
                                    tag="o")
                    for t in range(2):
                        nc.tensor.matmul(q_ps[:, :],
                                         wq_s[t][:, 128*ot:128*(ot+1)],
                                         x_t[t][:, 512*ih:512*(ih+1)],
                                         start=(t == 0), stop=(t == 1))
                    nc.scalar.activation(
                        out=qk_sb[ot][:, 512*ih:512*(ih+1)],
                        in_=q_ps[:, :], func=AF.Identity,
                        bias=bias_sb[:, ot:ot+1].bitcast(F32), scale=1.0)

                def vt_unit(nt):
                    vt = vT_sb[nt]
                    nc.gpsimd.memset(vt[:, :, 64:65], 1.0)
                    vt_ps = pso.tile([128, 256], F32, name=f"vps{sfx}n{nt}", tag="o")
                    for t in range(2):
                        nc.tensor.matmul(vt_ps[:, :],
                                         x_t[t][:, 128*nt:128*(nt+1)],
                                         wq_s[t][:, 512:768],
                                         start=(t == 0), stop=(t == 1))
                    nc.scalar.activation(
                        out=vt[:, :, 0:64],
                        in_=vt_ps[:, :].rearrange("p (h d) -> p h d", h=4),
                        func=AF.Copy)

                # pair-0 q/k needed before this batch's first score matmul
                for ot in (0, 2):
                    for ih in range(2):
                        qk_unit(ot, ih)
                # pair-1 q/k + all vt deferred into the p0 slot's jt loop
                S["units"] = ([lambda ot=ot, ih=ih: qk_unit(ot, ih)
                               for ot in (1, 3) for ih in range(2)]
                              + [lambda nt=nt: vt_unit(nt) for nt in range(8)])
                S["qk"] = qk_sb
                S["vt"] = vT_sb
                S["pts"] = {h: {} for h in range(4)}
                S["oall"] = [oap.tile([128, 1024], BF16, name=f"oall{sfx}t{t}",
                                      tag=f"oall{t}") for t in range(2)]
                return S

            def emit_scores(S, pair, jt):
                sfx = S["sfx"]
                qk_sb = S["qk"]
                s_ps = {}
                for hh in range(2):
                    h = 2 * pair + hh
                    s_ps[hh] = pss.tile([128, 1024], F32, name=f"sps{sfx}h{h}j{jt}",
                                        tag="s")
                for ih in range(2):
                    for hh in range(2):
                        nc.tensor.matmul(
                            s_ps[hh][:, 512*ih:512*(ih+1)],
                            qk_sb[2 + pair][64*hh:64*hh+64, 128*jt:128*(jt+1)],
                            qk_sb[pair][64*hh:64*hh+64, 512*ih:512*(ih+1)],
                            start=True, stop=True,
                            tile_position=(64*hh, 0))
                for hh in range(2):
                    h = 2 * pair + hh
                    pt = ptp.tile([128, 1024], BF16, name=f"pt{sfx}h{h}j{jt}", tag="pt")
                    if hh == 1 and jt in (0, 2, 3, 5, 6):
                        # Schraudolph on DVE: bf16 bits = round(A*s + B)
                        nc.vector.tensor_scalar(out=pt[:, :].bitcast(I16),
                                                in0=s_ps[hh][:, :],
                                                scalar1=SCH_A, scalar2=SCH_B,
                                                op0=ALU.mult, op1=ALU.add)
                    elif hh == 0 and jt in (1, 4, 6):
                        # hybrid: ACT stages scaled scores to sbuf bf16,
                        # Pool does the Schraudolph affine
                        sx = sxp.tile([128, 1024], BF16, name=f"sx{sfx}h{h}j{jt}",
                                      tag="sx")
                        nc.scalar.activation(out=sx[:, :], in_=s_ps[hh][:, :],
                                             func=AF.Identity, scale=0.125)
                        nc.gpsimd.tensor_scalar(out=pt[:, :].bitcast(I16),
                                                in0=sx[:, :],
                                                scalar1=8.0 * SCH_A, scalar2=SCH_B,
                                                op0=ALU.mult, op1=ALU.add)
                    else:
                        nc.scalar.activation(out=pt[:, :], in_=s_ps[hh][:, :],
                                             func=AF.Exp, scale=0.125)
                    S["pts"][h][jt] = pt

            def emit_attnv_norm(S, pair, u):
                sfx = S["sfx"]
                pts = S["pts"]
                if u % 2 == 0:
                    S[f"tr{pair}g{u//2}"] = pst.tile(
                        [128, 512], BF16, name=f"tr{sfx}p{pair}g{u//2}", tag="tr")
                tr_ps = S[f"tr{pair}g{u//2}"]
                o_ps = pso.tile([128, 512], F32, name=f"ops{sfx}p{pair}u{u}", tag="o")
                for g in range(4):          # g = 2*(it-2u) + hh
                    it = 2 * u + g // 2
                    hh = g % 2
                    h = 2 * pair + hh
                    for jt in range(8):
                        nc.tensor.matmul(
                            o_ps[:, 66*g:66*g+65],
                            pts[h][jt][:, 128*it:128*(it+1)],
                            S["vt"][jt][:, h, :],
                            start=(g == 0 and jt == 0), stop=(jt == 7),
                            skip_group_check=(g > 0 and jt == 0))
                rec = smallp.tile([128, 4], F32, name=f"rec{sfx}p{pair}u{u}",
                                  tag="rec")
                nc.vector.reciprocal_approx_fast(
                    out=rec[:, :].rearrange("p (g c) -> p g c", c=1),
                    in_=o_ps[:, 0:264].rearrange("p (g c) -> p g c", g=4)[:, :, 64:65])
                for k in range(2):          # k = it - 2u
                    it = 2 * u + k
                    ut = sxp.tile([128, 128], BF16, name=f"u{sfx}p{pair}i{it}",
                                  tag="u")
                    for hh in range(2):
                        g = 2 * k + hh
                        if hh == 0:
                            nc.vector.tensor_scalar(
                                out=ut[:, 0:64], in0=o_ps[:, 66*g:66*g+64],
                                scalar1=rec[:, g:g+1], scalar2=None, op0=ALU.mult)
                        else:
                            nc.scalar.activation(
                                out=ut[:, 64:128], in_=o_ps[:, 66*g:66*g+64],
                                func=AF.Copy, scale=rec[:, g:g+1])
                    nc.tensor.transpose(
                        out=tr_ps[:, 128*(it % 4):128*(it % 4)+128],
                        in_=ut[:, :], identity=ident_t[:, :])
                if u % 2 == 1:
                    nc.scalar.activation(
                        out=S["oall"][pair][:, 512*(u//2):512*(u//2)+512],
                        in_=tr_ps[:, :], func=AF.Copy)

            def emit_proj(S):
                sfx = S["sfx"]
                b = S["b"]
                y_sb = [yp.tile([128, 1024], F32, name=f"y{sfx}t{t}", tag=f"y{t}")
                        for t in range(2)]
                for ot in range(2):
                    for ih in range(2):
                        p_ps = pso.tile([128, 512], F32, name=f"pps{sfx}o{ot}i{ih}",
                                        tag="o")
                        for t in range(2):
                            nc.tensor.matmul(p_ps[:, :],
                                             projT_t[t][:, 128*ot:128*(ot+1)],
                                             S["oall"][t][:, 512*ih:512*(ih+1)],
                                             start=(t == 0), stop=(t == 1))
                        nc.vector.affine_then_add(
                            out=y_sb[ot][:, 512*ih:512*(ih+1)], in0=p_ps[:, :],
                            in1=S["x"][ot][:, 512*ih:512*(ih+1)].bitcast(F32),
                            scale=1.0, bias=S["pbias"][:, ot:ot+1])
                for ot in range(2):
                    nc.sync.dma_start(out=y_ap[b, 128*ot:128*(ot+1), :],
                                      in_=y_sb[ot][:, :])

            unroll = 1
            if loop_reps > 1:
                for u in (4, 2):
                    if loop_reps % u == 0:
                        unroll = u
                        break
            loop_ctx = (tc.For_i(0, loop_reps // unroll, 1,
                                 hint_engines=(mybir.EngineType.PE,))
                        if loop_reps > 1 else contextlib.nullcontext())
            with loop_ctx:
                S_by_b = {}
                prev = None
                for rep in range(unroll):
                    slots = [(b, p) for b in range(nbatch) for p in range(2)]
                    for (b, pair) in slots:
                        if pair == 0:
                            S_by_b[b] = emit_prelude(b, f"r{rep}b{b}")
                        units = S_by_b[b].get("units", []) if pair == 0 else []
                        for jt in range(8):
                            emit_scores(S_by_b[b], pair, jt)
                            for _ in range(2):
                                if units:
                                    units.pop(0)()
                            if prev is not None and jt % 2 == 1:
                                pb, pp = prev
                                blk = jt // 2
                                emit_attnv_norm(S_by_b[pb], pp, blk)
                                if blk == 3 and pp == 1:
                                    emit_proj(S_by_b[pb])
                        prev = (b, pair)
                        if pair == 0:
                            S_by_b[b]["units"] = units
                pb, pp = prev
                for blk in range(4):
                    emit_attnv_norm(S_by_b[pb], pp, blk)
                emit_proj(S_by_b[pb])

    nc.compile()
    return nc


def host_constants(gn_w, gn_b, qkv_w, qkv_b, proj_w, proj_b):
    """Fold static parameters into the tensors the kernel expects."""
    import ml_dtypes
    wqkv = qkv_w * gn_w[None, :]             # [768, 256]
    cbeta = qkv_w @ gn_b + qkv_b             # [768]
    sel = np.zeros((2, 128, 32), np.float32)
    selT = np.zeros((2, 32, 128), np.float32)
    for t in range(2):
        for c in range(128):
            g = (128 * t + c) // 8
            sel[t, c, g] = 1.0
            selT[t, g, c] = 1.0
    return {
        "wqkvT": np.ascontiguousarray(wqkv.T).astype(np.float32),   # [256, 768]
        "projT": np.ascontiguousarray(proj_w.T).astype(ml_dtypes.bfloat16),
        "cbeta": np.ascontiguousarray(cbeta.reshape(6, 128).T).astype(np.float32),
        "cproj": np.ascontiguousarray(proj_b.reshape(2, 128).T).astype(np.float32),
        "sel": sel, "selT": selT,
    }


_CACHE = {}


def kernel(x, gn_w, gn_b, qkv_w, qkv_b, proj_w, proj_b):
    from concourse.bass_utils import run_bass_kernel_spmd

    x = np.asarray(x, dtype=np.float32)
    consts = host_constants(np.asarray(gn_w, np.float32), np.asarray(gn_b, np.float32),
                            np.asarray(qkv_w, np.float32), np.asarray(qkv_b, np.float32),
                            np.asarray(proj_w, np.float32), np.asarray(proj_b, np.float32))
    if "nc" not in _CACHE:
        _CACHE["nc"] = build()
    nc = _CACHE["nc"]
    n_cores = 8
    bpc = x.shape[0] // n_cores
    in_maps = [{"x": np.ascontiguousarray(x[bpc*i:bpc*(i+1)]), **consts}
               for i in range(n_cores)]
    res = run_bass_kernel_spmd(nc, in_maps, core_ids=list(range(n_cores)))
    return np.concatenate([res.results[i]["y"] for i in range(n_cores)],
                          axis=0).astype(np.float32)



# revision 4
# speedup vs baseline: 1.0622x; 1.0052x over previous
"""V2.2: engine-rebalanced AttentionBlock kernel + swapped attn@v + fast rsqrt.

Changes vs baseline:
- softmax exp split: ~2/3 of tiles exact exp on ACT, ~1/3 Schraudolph
  bit-trick (f32->int16 affine, bitcast bf16) on DVE
- qk bias-add + vt psum->sbuf copies moved to ACT (activation Copy)
- qkv weight scaling + attn-normalize mult moved to Pool (gpsimd)
- vt ones-column init via Pool memset instead of DMA
"""
import contextlib
import numpy as np
import concourse.bacc as bacc
import concourse.tile as tile
from concourse import mybir, masks

F32 = mybir.dt.float32
F32R = mybir.dt.float32r
BF16 = mybir.dt.bfloat16
I16 = mybir.dt.int16
AF = mybir.ActivationFunctionType
ALU = mybir.AluOpType

B_PER_CORE = 2
EPS = 1e-5

# Schraudolph exp: exp(0.125*s) ~= bf16_bits(round(A*s + B))
LOG2E = 1.4426950408889634
SCH_A = 0.125 * 128.0 * LOG2E
SCH_B = 127.0 * 128.0 - 11.0


def build(nbatch=B_PER_CORE, loop_reps=1):
    nc = bacc.Bacc("TRN2", target_bir_lowering=False, debug=False)

    x_d = nc.dram_tensor("x", [nbatch, 256, 32, 32], F32R, kind="ExternalInput")
    wqkvT_d = nc.dram_tensor("wqkvT", [256, 768], F32R, kind="ExternalInput")
    projT_d = nc.dram_tensor("projT", [256, 256], BF16, kind="ExternalInput")
    cbeta_d = nc.dram_tensor("cbeta", [128, 6], F32, kind="ExternalInput")
    cproj_d = nc.dram_tensor("cproj", [128, 2], F32, kind="ExternalInput")
    sel_d = nc.dram_tensor("sel", [2, 128, 32], F32R, kind="ExternalInput")
    selT_d = nc.dram_tensor("selT", [2, 32, 128], F32R, kind="ExternalInput")
    y_d = nc.dram_tensor("y", [nbatch, 256, 32, 32], F32, kind="ExternalOutput")

    x_ap = x_d.ap().rearrange("b c h w -> b c (h w)")
    y_ap = y_d.ap().rearrange("b c h w -> b c (h w)")

    with tile.TileContext(nc) as tc:
        with tc.tile_pool(name="const", bufs=1) as constp, \
             tc.tile_pool(name="xp", bufs=2) as xp, \
             tc.tile_pool(name="wsp", bufs=2) as wsp, \
             tc.tile_pool(name="qkp", bufs=2) as qkp, \
             tc.tile_pool(name="vtp", bufs=2) as vtp, \
             tc.tile_pool(name="ptp", bufs=32) as ptp, \
             tc.tile_pool(name="sxp", bufs=3) as sxp, \
             tc.tile_pool(name="oap", bufs=2) as oap, \
             tc.tile_pool(name="smallp", bufs=4) as smallp, \
             tc.tile_pool(name="yp", bufs=2) as yp, \
             tc.tile_pool(name="pss", bufs=2, space="PSUM") as pss, \
             tc.tile_pool(name="pso", bufs=3, space="PSUM") as pso, \
             tc.tile_pool(name="pst", bufs=1, space="PSUM") as pst:

            # ---- constants (loaded once) ----
            wqkvT_t = [constp.tile([128, 768], F32R, name=f"wqkvT{t}") for t in range(2)]
            projT_t = [constp.tile([128, 256], BF16, name=f"projT{t}") for t in range(2)]
            cbeta_t = constp.tile([128, 6], F32)
            cproj_t = constp.tile([128, 2], F32)
            sel_t = [constp.tile([128, 32], F32R, name=f"sel{t}") for t in range(2)]
            selT_t = [constp.tile([32, 128], F32R, name=f"selT{t}") for t in range(2)]
            ident_t = constp.tile([128, 128], BF16)
            for t in range(2):
                nc.sync.dma_start(out=wqkvT_t[t][:, :], in_=wqkvT_d.ap()[128*t:128*(t+1), :])
                nc.sync.dma_start(out=projT_t[t][:, :], in_=projT_d.ap()[128*t:128*(t+1), :])
                nc.sync.dma_start(out=sel_t[t][:, :], in_=sel_d.ap()[t, :, :])
                nc.sync.dma_start(out=selT_t[t][:, :], in_=selT_d.ap()[t, :, :])
            nc.sync.dma_start(out=cbeta_t[:, :], in_=cbeta_d.ap()[:, :])
            nc.sync.dma_start(out=cproj_t[:, :], in_=cproj_d.ap()[:, :])
            masks.make_identity(nc, ident_t[:, :])

            def emit_prelude(b, sfx):
                S = {"b": b, "sfx": sfx}
                x_t = [xp.tile([128, 1024], F32R, name=f"x{sfx}t{t}", tag=f"x{t}")
                       for t in range(2)]
                for t in range(2):
                    nc.sync.dma_start(out=x_t[t][:, :], in_=x_ap[b, 128*t:128*(t+1), :])
                S["x"] = x_t

                # GroupNorm stats
                m2mv = []
                for t in range(2):
                    stats = smallp.tile([128, 2, 6], F32, name=f"st{sfx}t{t}", tag="stats")
                    xf = x_t[t][:, :].bitcast(F32).rearrange("p (s n) -> p s n", s=2)
                    nc.vector.bn_stats(out=stats[:, 0, :], in_=xf[:, 0, :])
                    nc.vector.bn_stats(out=stats[:, 1, :], in_=xf[:, 1, :])
                    mv = smallp.tile([128, 2], F32, name=f"mv{sfx}t{t}", tag="mv")
                    nc.vector.bn_aggr(out=mv[:, :], in_=stats[:, :, :])
                    mm = smallp.tile([128, 2], F32R, name=f"mm{sfx}t{t}", tag="mm")
                    nc.vector.tensor_copy(out=mm[:, 0:1], in_=mv[:, 0:1])
                    nc.vector.tensor_scalar(out=mm[:, 1:2], in0=mv[:, 0:1],
                                            scalar1=mv[:, 0:1], scalar2=mv[:, 1:2],
                                            op0=ALU.mult, op1=ALU.add)
                    m2mv.append(mm)
                gstat_ps = pso.tile([32, 512], F32, name=f"gst{sfx}", tag="o")
                for t in range(2):
                    nc.tensor.matmul(gstat_ps[:, 0:2], sel_t[t][:, :].bitcast(F32),
                                     m2mv[t][:, :].bitcast(F32),
                                     start=(t == 0), stop=(t == 1))

                # group mean / rstd (Newton rsqrt; var ~ 1)
                gmu = smallp.tile([32, 2], F32R, name=f"gmu{sfx}", tag="gmu")
                nc.vector.tensor_scalar(out=gmu[:, 0:1], in0=gstat_ps[:, 0:1],
                                        scalar1=0.125, scalar2=None, op0=ALU.mult)
                ta = smallp.tile([32, 4], F32, name=f"ta{sfx}", tag="ta")
                nc.vector.tensor_scalar(out=ta[:, 0:1], in0=gstat_ps[:, 1:2],
                                        scalar1=0.125, scalar2=EPS,
                                        op0=ALU.mult, op1=ALU.add)
                gmuf = gmu[:, 0:1].bitcast(F32)
                nc.vector.tensor_scalar(out=ta[:, 1:2], in0=gmuf, scalar1=gmuf,
                                        scalar2=None, op0=ALU.mult)
                nc.vector.tensor_tensor(out=ta[:, 2:3], in0=ta[:, 0:1], in1=ta[:, 1:2],
                                        op=ALU.subtract)
                nc.scalar.activation(out=ta[:, 3:4], in_=ta[:, 2:3], func=AF.Sqrt)
                nc.vector.reciprocal_approx_fast(out=gmu[:, 1:2].bitcast(F32),
                                                 in_=ta[:, 3:4])

                # broadcast (mu, rstd) to channels; scale W (Pool); biases
                chs = []
                for t in range(2):
                    ch_ps = pso.tile([128, 512], F32, name=f"chp{sfx}t{t}", tag="o")
                    nc.tensor.matmul(ch_ps[:, 0:2], selT_t[t][:, :].bitcast(F32),
                                     gmu[:, :].bitcast(F32), start=True, stop=True)
                    ch = smallp.tile([128, 2], F32R, name=f"chs{sfx}t{t}", tag="chs")
                    nc.vector.tensor_copy(out=ch[:, :], in_=ch_ps[:, 0:2])
                    chs.append(ch)
                wq_s = []
                for t in range(2):
                    ws = wsp.tile([128, 768], F32R, name=f"ws{sfx}t{t}", tag=f"ws{t}")
                    nc.vector.tensor_scalar(out=ws[:, :],
                                            in0=wqkvT_t[t][:, :].bitcast(F32),
                                            scalar1=chs[t][:, 1:2].bitcast(F32),
                                            scalar2=None, op0=ALU.mult)
                    wq_s.append(ws)
                bias_ps = pso.tile([128, 512], F32, name=f"bps{sfx}", tag="o")
                for h in range(6):
                    for t in range(2):
                        nc.tensor.matmul(bias_ps[:, h:h+1],
                                         wq_s[t][:, 128*h:128*(h+1)].bitcast(F32),
                                         chs[t][:, 0:1].bitcast(F32),
                                         start=(t == 0), stop=(t == 1),
                                         skip_group_check=(h > 0 and t == 0))
                bias_sb = smallp.tile([128, 6], F32R, name=f"bsb{sfx}", tag="bsb")
                nc.vector.tensor_tensor(out=bias_sb[:, :], in0=cbeta_t[:, :],
                                        in1=bias_ps[:, 0:6], op=ALU.subtract)
                # proj-side correction for the v bias
                bias16 = smallp.tile([128, 2], BF16, name=f"b16{sfx}", tag="b16")
                nc.vector.tensor_copy(out=bias16[:, :], in_=bias_sb[:, 4:6].bitcast(F32))
                pb_ps = pso.tile([128, 512], F32, name=f"pbps{sfx}", tag="o")
                for ot in range(2):
                    for t in range(2):
                        nc.tensor.matmul(pb_ps[:, ot:ot+1],
                                         projT_t[t][:, 128*ot:128*(ot+1)],
                                         bias16[:, t:t+1],
                                         start=(t == 0), stop=(t == 1),
                                         skip_group_check=(ot == 1 and t == 0))
                pbias_sb = smallp.tile([128, 2], F32, name=f"pbias{sfx}", tag="pbias")
                nc.vector.tensor_tensor(out=pbias_sb[:, :], in0=cproj_t[:, :],
                                        in1=pb_ps[:, 0:2], op=ALU.add)
                S["pbias"] = pbias_sb

                # qkv production units: emitted lazily so the ACT copies
                # spread across the first slot's score stream instead of one
                # head-of-line burst in the ACT FIFO
                qk_sb = [qkp.tile([128, 1024], BF16, name=f"qk{sfx}o{ot}",
                                  tag=f"qk{ot}") for ot in range(4)]
                vT_sb = [vtp.tile([128, 4, 65], BF16, name=f"vt{sfx}n{nt}",
                                  tag=f"vt{nt}") for nt in range(8)]

                def qk_unit(ot, ih):
                    q_ps = pso.tile([128, 512], F32, name=f"qps{sfx}o{ot}i{ih}",
                                    tag="o")
                    for t in range(2):
                        nc.tensor.matmul(q_ps[:, :],
                                         wq_s[t][:, 128*ot:128*(ot+1)],
                                         x_t[t][:, 512*ih:512*(ih+1)],
                                         start=(t == 0), stop=(t == 1))
                    nc.scalar.activation(
                        out=qk_sb[ot][:, 512*ih:512*(ih+1)],
                        in_=q_ps[:, :], func=AF.Identity,
                        bias=bias_sb[:, ot:ot+1].bitcast(F32), scale=1.0)

                def vt_unit(nt):
                    vt = vT_sb[nt]
                    nc.gpsimd.memset(vt[:, :, 64:65], 1.0)
                    vt_ps = pso.tile([128, 256], F32, name=f"vps{sfx}n{nt}", tag="o")
                    for t in range(2):
                        nc.tensor.matmul(vt_ps[:, :],
                                         x_t[t][:, 128*nt:128*(nt+1)],
                                         wq_s[t][:, 512:768],
                                         start=(t == 0), stop=(t == 1))
                    nc.scalar.activation(
                        out=vt[:, :, 0:64],
                        in_=vt_ps[:, :].rearrange("p (h d) -> p h d", h=4),
                        func=AF.Copy)

                # pair-0 q/k needed before this batch's first score matmul
                for ot in (0, 2):
                    for ih in range(2):
                        qk_unit(ot, ih)
                # pair-1 q/k + all vt deferred into the p0 slot's jt loop
                S["units"] = ([lambda ot=ot, ih=ih: qk_unit(ot, ih)
                               for ot in (1, 3) for ih in range(2)]
                              + [lambda nt=nt: vt_unit(nt) for nt in range(8)])
                S["qk"] = qk_sb
                S["vt"] = vT_sb
                S["pts"] = {h: {} for h in range(4)}
                S["oall"] = [oap.tile([128, 1024], BF16, name=f"oall{sfx}t{t}",
                                      tag=f"oall{t}") for t in range(2)]
                return S

            def emit_scores(S, pair, jt):
                sfx = S["sfx"]
                qk_sb = S["qk"]
                s_ps = {}
                for hh in range(2):
                    h = 2 * pair + hh
                    s_ps[hh] = pss.tile([128, 1024], F32, name=f"sps{sfx}h{h}j{jt}",
                                        tag="s")
                for ih in range(2):
                    for hh in range(2):
                        nc.tensor.matmul(
                            s_ps[hh][:, 512*ih:512*(ih+1)],
                            qk_sb[2 + pair][64*hh:64*hh+64, 128*jt:128*(jt+1)],
                            qk_sb[pair][64*hh:64*hh+64, 512*ih:512*(ih+1)],
                            start=True, stop=True,
                            tile_position=(64*hh, 0))
                for hh in range(2):
                    h = 2 * pair + hh
                    pt = ptp.tile([128, 1024], BF16, name=f"pt{sfx}h{h}j{jt}", tag="pt")
                    if (hh, jt) in {(1, 0), (1, 2), (1, 6)}:
                        # Schraudolph on DVE: bf16 bits = round(A*s + B)
                        nc.vector.tensor_scalar(out=pt[:, :].bitcast(I16),
                                                in0=s_ps[hh][:, :],
                                                scalar1=SCH_A, scalar2=SCH_B,
                                                op0=ALU.mult, op1=ALU.add)
                    elif (hh, jt) in {(0, 1), (0, 4), (0, 6), (1, 3), (1, 5)}:
                        # hybrid: ACT stages scaled scores to sbuf bf16,
                        # Pool does the Schraudolph affine
                        sx = sxp.tile([128, 1024], BF16, name=f"sx{sfx}h{h}j{jt}",
                                      tag="sx")
                        nc.scalar.activation(out=sx[:, :], in_=s_ps[hh][:, :],
                                             func=AF.Identity, scale=0.125)
                        nc.gpsimd.tensor_scalar(out=pt[:, :].bitcast(I16),
                                                in0=sx[:, :],
                                                scalar1=8.0 * SCH_A, scalar2=SCH_B,
                                                op0=ALU.mult, op1=ALU.add)
                    else:
                        nc.scalar.activation(out=pt[:, :], in_=s_ps[hh][:, :],
                                             func=AF.Exp, scale=0.125)
                    S["pts"][h][jt] = pt

            def emit_attnv_norm(S, pair, u):
                sfx = S["sfx"]
                pts = S["pts"]
                if u % 2 == 0:
                    S[f"tr{pair}g{u//2}"] = pst.tile(
                        [128, 512], BF16, name=f"tr{sfx}p{pair}g{u//2}", tag="tr")
                tr_ps = S[f"tr{pair}g{u//2}"]
                o_ps = pso.tile([128, 512], F32, name=f"ops{sfx}p{pair}u{u}", tag="o")
                for g in range(4):          # g = 2*(it-2u) + hh
                    it = 2 * u + g // 2
                    hh = g % 2
                    h = 2 * pair + hh
                    for jt in range(8):
                        nc.tensor.matmul(
                            o_ps[:, 66*g:66*g+65],
                            pts[h][jt][:, 128*it:128*(it+1)],
                            S["vt"][jt][:, h, :],
                            start=(g == 0 and jt == 0), stop=(jt == 7),
                            skip_group_check=(g > 0 and jt == 0))
                rec = smallp.tile([128, 4], F32, name=f"rec{sfx}p{pair}u{u}",
                                  tag="rec")
                nc.vector.reciprocal_approx_fast(
                    out=rec[:, :].rearrange("p (g c) -> p g c", c=1),
                    in_=o_ps[:, 0:264].rearrange("p (g c) -> p g c", g=4)[:, :, 64:65])
                for k in range(2):          # k = it - 2u
                    it = 2 * u + k
                    ut = sxp.tile([128, 128], BF16, name=f"u{sfx}p{pair}i{it}",
                                  tag="u")
                    for hh in range(2):
                        g = 2 * k + hh
                        if hh == 0:
                            nc.vector.tensor_scalar(
                                out=ut[:, 0:64], in0=o_ps[:, 66*g:66*g+64],
                                scalar1=rec[:, g:g+1], scalar2=None, op0=ALU.mult)
                        else:
                            nc.scalar.activation(
                                out=ut[:, 64:128], in_=o_ps[:, 66*g:66*g+64],
                                func=AF.Copy, scale=rec[:, g:g+1])
                    nc.tensor.transpose(
                        out=tr_ps[:, 128*(it % 4):128*(it % 4)+128],
                        in_=ut[:, :], identity=ident_t[:, :])
                if u % 2 == 1:
                    nc.scalar.activation(
                        out=S["oall"][pair][:, 512*(u//2):512*(u//2)+512],
                        in_=tr_ps[:, :], func=AF.Copy)

            def emit_proj(S):
                sfx = S["sfx"]
                b = S["b"]
                y_sb = [yp.tile([128, 1024], F32, name=f"y{sfx}t{t}", tag=f"y{t}")
                        for t in range(2)]
                for ot in range(2):
                    for ih in range(2):
                        p_ps = pso.tile([128, 512], F32, name=f"pps{sfx}o{ot}i{ih}",
                                        tag="o")
                        for t in range(2):
                            nc.tensor.matmul(p_ps[:, :],
                                             projT_t[t][:, 128*ot:128*(ot+1)],
                                             S["oall"][t][:, 512*ih:512*(ih+1)],
                                             start=(t == 0), stop=(t == 1))
                        nc.vector.affine_then_add(
                            out=y_sb[ot][:, 512*ih:512*(ih+1)], in0=p_ps[:, :],
                            in1=S["x"][ot][:, 512*ih:512*(ih+1)].bitcast(F32),
                            scale=1.0, bias=S["pbias"][:, ot:ot+1])
                for ot in range(2):
                    nc.sync.dma_start(out=y_ap[b, 128*ot:128*(ot+1), :],
                                      in_=y_sb[ot][:, :])

            unroll = 1
            if loop_reps > 1:
                for u in (4, 2):
                    if loop_reps % u == 0:
                        unroll = u
                        break
            loop_ctx = (tc.For_i(0, loop_reps // unroll, 1,
                                 hint_engines=(mybir.EngineType.PE,))
                        if loop_reps > 1 else contextlib.nullcontext())
            with loop_ctx:
                S_by_b = {}
                prev = None
                for rep in range(unroll):
                    slots = [(b, p) for b in range(nbatch) for p in range(2)]
                    for (b, pair) in slots:
                        if pair == 0:
                            S_by_b[b] = emit_prelude(b, f"r{rep}b{b}")
                        units = S_by_b[b].get("units", []) if pair == 0 else []
                        for jt in range(8):
                            emit_scores(S_by_b[b], pair, jt)
                            for _ in range(2):
                                if units:
                                    units.pop(0)()
                            if prev is not None and jt % 2 == 1:
                                pb, pp = prev
                                blk = jt // 2
                                emit_attnv_norm(S_by_b[pb], pp, blk)
                                if blk == 3 and pp == 1:
                                    emit_proj(S_by_b[pb])
                        prev = (b, pair)
                        if pair == 0:
                            S_by_b[b]["units"] = units
                pb, pp = prev
                for blk in range(4):
                    emit_attnv_norm(S_by_b[pb], pp, blk)
                emit_proj(S_by_b[pb])

    nc.compile()
    return nc


def host_constants(gn_w, gn_b, qkv_w, qkv_b, proj_w, proj_b):
    """Fold static parameters into the tensors the kernel expects."""
    import ml_dtypes
    wqkv = qkv_w * gn_w[None, :]             # [768, 256]
    cbeta = qkv_w @ gn_b + qkv_b             # [768]
    sel = np.zeros((2, 128, 32), np.float32)
    selT = np.zeros((2, 32, 128), np.float32)
    for t in range(2):
        for c in range(128):
            g = (128 * t + c) // 8
            sel[t, c, g] = 1.0
            selT[t, g, c] = 1.0
    return {
        "wqkvT": np.ascontiguousarray(wqkv.T).astype(np.float32),   # [256, 768]
        "projT": np.ascontiguousarray(proj_w.T).astype(ml_dtypes.bfloat16),
        "cbeta": np.ascontiguousarray(cbeta.reshape(6, 128).T).astype(np.float32),
        "cproj": np.ascontiguousarray(proj_b.reshape(2, 128).T).astype(np.float32),
        "sel": sel, "selT": selT,
    }


_CACHE = {}


def kernel(x, gn_w, gn_b, qkv_w, qkv_b, proj_w, proj_b):
    from concourse.bass_utils import run_bass_kernel_spmd

    x = np.asarray(x, dtype=np.float32)
    consts = host_constants(np.asarray(gn_w, np.float32), np.asarray(gn_b, np.float32),
                            np.asarray(qkv_w, np.float32), np.asarray(qkv_b, np.float32),
                            np.asarray(proj_w, np.float32), np.asarray(proj_b, np.float32))
    if "nc" not in _CACHE:
        _CACHE["nc"] = build()
    nc = _CACHE["nc"]
    n_cores = 8
    bpc = x.shape[0] // n_cores
    in_maps = [{"x": np.ascontiguousarray(x[bpc*i:bpc*(i+1)]), **consts}
               for i in range(n_cores)]
    res = run_bass_kernel_spmd(nc, in_maps, core_ids=list(range(n_cores)))
    return np.concatenate([res.results[i]["y"] for i in range(n_cores)],
                          axis=0).astype(np.float32)

